# revision 1
# baseline (speedup 1.0000x reference)
"""Mixtral sparse MoE block on 8 Trainium2 NeuronCores.

Strategy: expert-parallel, single dispatch. Each of the 8 cores owns one
expert's weights (w1[e], w2[e], w3[e]). The host routes tokens: for each
expert, gather the tokens that selected it into a capacity-C buffer
(C = max expert membership, rounded up; 528 for the reference input),
ship the gathered tokens transposed plus the expert's weights, and the
core computes yT = (silu(xg @ w1) * (xg @ w2)) @ w3 transposed, densely
over the C token slots. The host applies the routing gates and
scatter-adds each expert's rows back into the full [T, H] output.

All matmuls run in bf16 with fp32 PSUM accumulation (~2e-3 relative
error, well inside the 2e-2 gate). bf16 halves HBM traffic vs fp32r
(48MB of weights per core instead of 96MB), which removes every
DMA-induced PE stall, and enables FWL fast weight loads.

Since C=528 exceeds one PSUM bank (512 fp32), each logical [128, C]
output tile is split into two column blocks (272 + C-272) living in two
PSUM banks; every weight chunk issues two matmuls back-to-back (the
second LDWEIGHTS hides under the first matmul's 113ns stream).

All DRAM tensors are pre-tiled on the host into per-partition-contiguous
layouts so every DMA descriptor moves multi-KB contiguous runs.
"""

import sys

for _p in ("/opt/trn_rl_repo", "/root/.axon_site/_ro/trn_rl_repo"):
    if _p not in sys.path:
        sys.path.append(_p)

import numpy as np

H = 2048  # hidden dim
F = 4096  # ffn dim
E = 8     # experts
HT = H // 128
FT = F // 128
C1 = 272  # first psum column block when cap > 512

_COMPILED = {}

# set by a driver (e.g. test.py) to profile the next dispatch
TRACE = False
LAST_EXEC_NS = None
LAST_RESULTS = None


def _ensure_ntff_hook():
    """Install antenv.axon_hooks shim + register the axon NTFF profile hook
    if the image's antenv package lacks it. Only needed for TRACE runs."""
    try:
        from antenv import axon_hooks  # noqa: F401
        return
    except ImportError:
        pass
    import types
    import antenv

    mod = types.ModuleType("antenv.axon_hooks")
    _hook = [None]
    mod.set_axon_ntff_profile_hook = lambda h: _hook.__setitem__(0, h)
    mod.get_axon_ntff_profile_hook = lambda: _hook[0]
    sys.modules["antenv.axon_hooks"] = mod
    antenv.axon_hooks = mod
    try:
        from trn_agent_boot.trn_boot import _ntff_profile_via_ctypes
        mod.set_axon_ntff_profile_hook(
            _ntff_profile_via_ctypes("/opt/axon/libaxon_pjrt.so")
        )
    except Exception:
        pass


def _build(cap: int):
    import concourse.bacc as bacc
    import concourse.tile as tile
    from concourse import mybir

    F32 = mybir.dt.float32
    BF16 = mybir.dt.bfloat16

    # column blocks per logical cap-wide tile (each must fit a PSUM bank)
    if cap <= 512:
        blocks = [(0, cap)]
    else:
        assert cap <= C1 + 512
        blocks = [(0, C1), (C1, cap)]

    nc = bacc.Bacc("TRN2", target_bir_lowering=False, debug=False, num_devices=E)
    # pre-tiled layouts (see kernel() for the host-side packing):
    #   xgT[p, t, c]     = x_gathered[c, t*128+p]
    #   w1[p, fc, t, j]  = w1_e[t*128+p, fc*128+j]   (w2 identical)
    #   w3[p, ht, fc, j] = w3_e[fc*128+p, ht*128+j]
    #   yT[p, t, c]      = y[c, t*128+p]
    xgT = nc.dram_tensor("xgT", [128, HT, cap], BF16, kind="ExternalInput").ap()
    w1 = nc.dram_tensor("w1", [128, FT, HT, 128], BF16, kind="ExternalInput").ap()
    w2 = nc.dram_tensor("w2", [128, FT, HT, 128], BF16, kind="ExternalInput").ap()
    w3 = nc.dram_tensor("w3", [128, HT, FT, 128], BF16, kind="ExternalInput").ap()
    yT = nc.dram_tensor("yT", [128, HT, cap], BF16, kind="ExternalOutput").ap()

    with tile.TileContext(nc) as tc:
        with (
            tc.tile_pool(name="resident", bufs=1) as resident,
            tc.tile_pool(name="wpool", bufs=2) as wpool,
            tc.tile_pool(name="w3pool", bufs=3) as w3pool,
            tc.tile_pool(name="spool", bufs=2) as spool,
            tc.tile_pool(name="ypool", bufs=2) as ypool,
            tc.tile_pool(name="ps", bufs=2, space="PSUM") as psp,
        ):
            xg_s = resident.tile([128, HT, cap], BF16)
            hT_s = resident.tile([128, FT, cap], BF16)

            # HAM warmup: dummy bf16 matmuls on a DVE-memset zeros tile run
            # while the first token/weight DMAs stream in (DMA-queue spin-up
            # plus ~4MB of upfront traffic keeps real data away until
            # ~15-20us), so the PE clock-gate is already released (2.4GHz)
            # and stays released when real matmuls start. ~16 cold MMs cover
            # the 3.4us HAM SHORT window; the rest bridge the DMA ramp.
            warm = resident.tile([128, 256], BF16)
            nc.gpsimd.memset(warm[:], 0.0)
            # one PSUM tile reused by every warm matmul: same-engine WAW
            # ordering is free, so the stream issues back-to-back (a fresh
            # pool tile per matmul serializes on buffer-rotation semaphores)
            # 28 MMs: memset lands ~7us, 16 cold MMs cover the 3.4us HAM
            # SHORT window, the rest bridge to the first token/weight DMA
            # completion (~11.3us — packets round-robin all 16 HW queues,
            # so the first-issued critical DMAs finish early)
            pw = psp.tile([128, 2048], F32, tag="ps")
            for i in range(48):
                nc.tensor.matmul(
                    pw[:, :256], warm[:, :128], warm[:], start=True, stop=True
                )

            # Phase A: hT[f] = silu(w1_f.T @ xg) * (w2_f.T @ xg)  (F on parts)
            for f in range(FT):
                w1c = wpool.tile([128, HT, 128], BF16, tag="w1c")
                w2c = wpool.tile([128, HT, 128], BF16, tag="w2c")
                if f == 0:
                    # ramp: first token quarter + first w1 chunks enable the
                    # first matmuls ASAP; the rest split into several
                    # descriptors (DMA queues round-robin across active
                    # descriptors, so finer splits land progressively and
                    # the group consumes chunks as they arrive)
                    nc.sync.dma_start(xg_s[:, :4, :], xgT[:, :4, :])
                    nc.sync.dma_start(w1c[:, :4], w1[:, f, :4])
                    nc.sync.dma_start(w1c[:, 4:], w1[:, f, 4:])
                    nc.sync.dma_start(xg_s[:, 4:8, :], xgT[:, 4:8, :])
                    nc.sync.dma_start(xg_s[:, 8:12, :], xgT[:, 8:12, :])
                    nc.sync.dma_start(xg_s[:, 12:, :], xgT[:, 12:, :])
                else:
                    nc.sync.dma_start(w1c[:], w1[:, f])
                nc.sync.dma_start(w2c[:], w2[:, f])

                ps = psp.tile([128, 2048], F32, tag="ps")
                # pa blocks at fp32 cols [0:512], pb blocks at [1024:1536+]
                for t in range(HT):
                    for bi, (lo, hi) in enumerate(blocks):
                        nc.tensor.matmul(
                            ps[:, 512 * bi: 512 * bi + (hi - lo)],
                            w1c[:, t, :], xg_s[:, t, lo:hi],
                            start=(t == 0), stop=(t == HT - 1),
                        )
                for t in range(HT):
                    for bi, (lo, hi) in enumerate(blocks):
                        nc.tensor.matmul(
                            ps[:, 1024 + 512 * bi: 1024 + 512 * bi + (hi - lo)],
                            w2c[:, t, :], xg_s[:, t, lo:hi],
                            start=(t == 0), stop=(t == HT - 1),
                        )
                sa = spool.tile([128, cap], F32, tag="sa")
                for bi, (lo, hi) in enumerate(blocks):
                    nc.scalar.activation(
                        sa[:, lo:hi], ps[:, 512 * bi: 512 * bi + (hi - lo)],
                        mybir.ActivationFunctionType.Silu,
                    )
                for bi, (lo, hi) in enumerate(blocks):
                    nc.vector.tensor_mul(
                        hT_s[:, f, lo:hi], sa[:, lo:hi],
                        ps[:, 1024 + 512 * bi: 1024 + 512 * bi + (hi - lo)],
                    )

            # Phase B: yT[ht] = sum_f w3_chunk(ht,f).T @ hT[f]  (H on parts)
            for t in range(HT):
                w3c = w3pool.tile([128, FT, 128], BF16, tag="w3c")
                nc.sync.dma_start(w3c[:, :FT // 2], w3[:, t, :FT // 2])
                nc.sync.dma_start(w3c[:, FT // 2:], w3[:, t, FT // 2:])
                ps = psp.tile([128, 2048], F32, tag="ps")
                for f in range(FT):
                    for bi, (lo, hi) in enumerate(blocks):
                        nc.tensor.matmul(
                            ps[:, 512 * bi: 512 * bi + (hi - lo)],
                            w3c[:, f, :], hT_s[:, f, lo:hi],
                            start=(f == 0), stop=(f == FT - 1),
                        )
                yt = ypool.tile([128, cap], BF16, tag="yt")
                for bi, (lo, hi) in enumerate(blocks):
                    nc.vector.tensor_copy(
                        yt[:, lo:hi], ps[:, 512 * bi: 512 * bi + (hi - lo)]
                    )
                # store split across the PARTITION dim: one DMA descriptor
                # drains at only ~16-21GB/s (single-queue share), which put
                # the last store's 8.5us squarely on the exec critical path;
                # four 32-partition descriptors keep full-length runs and
                # quadruple the queue parallelism (~2us)
                for q in range(4):
                    nc.sync.dma_start(
                        yT[32 * q: 32 * q + 32, t, :], yt[32 * q: 32 * q + 32, :]
                    )

    nc.compile()
    return nc


def _get_compiled(cap: int):
    if cap not in _COMPILED:
        _COMPILED[cap] = _build(cap)
    return _COMPILED[cap]


def kernel(hidden_states, selected_experts, routing_weights, w1, w2, w3):
    global LAST_EXEC_NS, LAST_RESULTS
    from concourse.bass_utils import run_bass_kernel_spmd
    import ml_dtypes

    BF = ml_dtypes.bfloat16

    hs = np.ascontiguousarray(np.asarray(hidden_states), dtype=np.float32)
    sel = np.asarray(selected_experts)
    rw = np.ascontiguousarray(np.asarray(routing_weights), dtype=np.float32)
    w1 = np.asarray(w1)
    w2 = np.asarray(w2)
    w3 = np.asarray(w3)

    T = hs.shape[0]
    K = sel.shape[1]
    assert hs.shape[1] == H and w1.shape == (E, H, F) and w3.shape == (E, F, H)

    # host routing: gate[t, e] = sum_k rw[t, k] * (sel[t, k] == e)
    gate = np.zeros((T, E), np.float32)
    member = np.zeros((T, E), bool)
    tix = np.arange(T)
    for k in range(K):
        np.add.at(gate, (tix, sel[:, k]), rw[:, k])
        member[tix, sel[:, k]] = True
    idx = [np.nonzero(member[:, e])[0] for e in range(E)]

    xr = hs.astype(BF)  # [T, H]
    # pre-tile weights into per-partition-contiguous layouts (bf16)
    w1p = [
        np.ascontiguousarray(
            w1[e].astype(BF).reshape(HT, 128, FT, 128).transpose(1, 2, 0, 3)
        )
        for e in range(E)
    ]
    w2p = [
        np.ascontiguousarray(
            w2[e].astype(BF).reshape(HT, 128, FT, 128).transpose(1, 2, 0, 3)
        )
        for e in range(E)
    ]
    w3p = [
        np.ascontiguousarray(
            w3[e].astype(BF).reshape(FT, 128, HT, 128).transpose(1, 2, 0, 3)
        )
        for e in range(E)
    ]

    if TRACE:
        _ensure_ntff_hook()
    maxc = max(len(i) for i in idx)
    # capacity: all member tokens of every expert in one dispatch
    cap = max(272, ((maxc + 15) // 16) * 16)
    assert cap <= C1 + 512, "unexpectedly unbalanced routing"
    nc = _get_compiled(cap)
    out = np.zeros((T, H), np.float32)
    in_maps = []
    for e in range(E):
        ii = idx[e]
        xgT = np.zeros((128, HT, cap), BF)
        if len(ii):
            # xgT[p, t, :n] = xr[ii][:, t*128+p].T
            xgT[:, :, :len(ii)] = (
                xr[ii].reshape(len(ii), HT, 128).transpose(2, 1, 0)
            )
        in_maps.append({
            "xgT": xgT,
            "w1": w1p[e],
            "w2": w2p[e],
            "w3": w3p[e],
        })
    res = run_bass_kernel_spmd(
        nc, in_maps, core_ids=list(range(E)),
        trace=TRACE, trace_cores=(list(range(E)) if TRACE else None),
    )
    if TRACE:
        LAST_EXEC_NS = res.exec_time_ns
        LAST_RESULTS = res
    for e in range(E):
        ii = idx[e]
        if not len(ii):
            continue
        yT = res.results[e]["yT"].astype(np.float32)  # [128, HT, cap]
        y = yT.transpose(2, 1, 0).reshape(cap, H)  # [cap, H]
        out[ii] += gate[ii, e:e + 1] * y[:len(ii)]
    return out

